# revision 2
# baseline (speedup 1.0000x reference)
"""AttentionHead kernel for Trainium2 (8 NeuronCores, data-parallel over batch).

v4: 5-stage software pipeline over 4-batch groups, ready-first emission.
  S0(m): q|k packed projection + drains; v projection (x^T stationary) + drain
  S1(m): score matmuls + additive causal into PSUM
  S2(m): exp
  S3(m): dropout muls
  S4(m): Z sums, out matmuls, reciprocal, normalize, store
Emission m issues S4(m-4), S3(m-3), S2(m-2), S1(m-1), S0(m): every
cross-engine dependency is at least one emission old, so no engine blocks
on another engine's current work.
"""

import numpy as np

B, T, C, H = 512, 256, 256, 64
N_CORES = 8
BP = B // N_CORES
OB = 8
NO = BP // OB              # 8 octets (DMA granularity)
NG = BP // 4               # 16 groups (pipeline granularity)
P_DROP = 0.25
SCALE = float(H) ** -0.5
NEG = -50.0

_CACHE = {}
_LABELS = {}


def _build_program():
    import concourse.mybir as mybir
    from concourse import bacc
    from concourse.tile import TileContext

    f32 = mybir.dt.float32
    bf16 = mybir.dt.bfloat16
    u8 = mybir.dt.uint8
    AF = mybir.ActivationFunctionType
    ALU = mybir.AluOpType

    nc = bacc.Bacc()

    xq_d = nc.dram_tensor("xq", [NO, 2, 128, OB, T], bf16, kind="ExternalInput")
    m0_d = nc.dram_tensor("m0", [NO, 128, OB, T], bf16, kind="ExternalInput")
    m1_d = nc.dram_tensor("m1", [NO, 128, OB, 128], u8, kind="ExternalInput")
    wqk_d = nc.dram_tensor("wqk", [2, 128, 128], bf16, kind="ExternalInput")
    wv_d = nc.dram_tensor("wv", [2, 128, H], bf16, kind="ExternalInput")
    y_d = nc.dram_tensor("y", [NO, 2, 128, OB * H], bf16, kind="ExternalOutput")
    cst_d = nc.dram_tensor("cst", [3, 128, 128], bf16, kind="ExternalInput")

    copy_func = getattr(AF, "Copy", None) or getattr(AF, "Identity")

    with TileContext(nc) as tc:
        with (
            tc.tile_pool(name="const", bufs=1) as cpool,
            tc.tile_pool(name="iox", bufs=3) as iox,
            tc.tile_pool(name="iom", bufs=5) as iom,
            tc.tile_pool(name="wk", bufs=5) as wkp,
            tc.tile_pool(name="pqk", bufs=1, space="PSUM") as pp_qk,
            tc.tile_pool(name="pw", bufs=3, space="PSUM") as pp_w,
            tc.tile_pool(name="pv", bufs=1, space="PSUM") as pp_v,
            tc.tile_pool(name="pz", bufs=1, space="PSUM") as pp_z,
            tc.tile_pool(name="pout", bufs=1, space="PSUM") as pp_out,
        ):
            # ---- constants -------------------------------------------------
            wqk = cpool.tile([128, 256], bf16, tag="wqk")
            nc.scalar.dma_start(
                wqk[:].rearrange("p (a m) -> p a m", a=2),
                wqk_d[:].rearrange("a p m -> p a m"))
            wv = cpool.tile([128, 2 * H], bf16, tag="wv")
            nc.scalar.dma_start(
                wv[:].rearrange("p (a h) -> p a h", a=2),
                wv_d[:].rearrange("a p h -> p a h"))

            wsut = cpool.tile([128, 128], bf16, tag="wsut")
            nc.scalar.dma_start(wsut[:], cst_d[0])
            id128 = cpool.tile([128, 128], bf16, tag="id128")
            nc.scalar.dma_start(id128[:], cst_d[1])
            ones1 = cpool.tile([128, 1], bf16, tag="ones1")
            nc.scalar.dma_start(ones1[:], cst_d[2, :, 0:1])

            st = {}
            io = {}

            def dma_oct(o):
                xt = iox.tile([128, 2 * OB * T], bf16, tag="xt")
                for a_ in range(2):
                    nc.sync.dma_start(
                        xt[:, a_ * 2048:(a_ + 1) * 2048].rearrange(
                            "p (b t) -> p b t", b=OB),
                        xq_d[o, a_])
                m0 = iom.tile([128, OB * T], bf16, tag="m0")
                nc.sync.dma_start(
                    m0[:].rearrange("s (b t) -> s b t", b=OB), m0_d[o])
                m1 = iom.tile([128, OB * 128], u8, tag="m1")
                nc.sync.dma_start(
                    m1[:].rearrange("s (b t) -> s b t", b=OB), m1_d[o])
                io[o] = (xt, m0, m1)

            def stage0(m):
                o, j = divmod(m, 2)
                xt, _, _ = io[o]
                pqk = pp_qk.tile([128, 1024], f32, tag="pqk")
                for p2 in range(2):
                    xcol = j * 1024 + p2 * 512
                    sl = pqk[:, p2 * 512:p2 * 512 + 512]
                    nc.tensor.matmul(sl, wqk[:, 0:128],
                                     xt[:, xcol:xcol + 512],
                                     start=True, stop=False)
                    nc.tensor.matmul(sl, wqk[:, 128:256],
                                     xt[:, 2048 + xcol:2048 + xcol + 512],
                                     start=False, stop=True)
                vps = pp_v.tile([128, 512], f32, tag="pv")
                for bl in range(4):
                    for sb in range(2):
                        oc = (bl * 2 + sb) * H
                        xc = (j * 4 + bl) * 256 + sb * 128
                        nc.tensor.matmul(vps[:, oc:oc + H],
                                         xt[:, xc:xc + 128],
                                         wv[:, 0:H], start=True, stop=False)
                        nc.tensor.matmul(vps[:, oc:oc + H],
                                         xt[:, 2048 + xc:2048 + xc + 128],
                                         wv[:, H:2 * H],
                                         start=False, stop=True)
                st[m] = {"pqk": pqk, "vps": vps}

            def stage0b(m):
                s_ = st[m]
                pqk = s_.pop("pqk")
                vps = s_.pop("vps")
                v2 = wkp.tile([128, 512], bf16, tag="v2")
                nc.vector.tensor_copy(v2[:], vps[:])
                qkf = wkp.tile([64, 2048], bf16, tag="qkf")
                nc.scalar.activation(qkf[:, 0:1024], pqk[0:64, :], copy_func)
                nc.vector.tensor_copy(qkf[:, 1024:2048], pqk[64:128, :])
                s_.update(qkf=qkf, v2=v2)

            def stage1(m):
                qkf = st[m]["qkf"]
                wt = []
                for _wi in range(3):
                    w_t = pp_w.tile([128, 512], f32, tag="w", name=f"w{_wi}")
                    wt.append(w_t)
                for bi in range(4):
                    q0 = qkf[:, bi * 256: bi * 256 + 128]
                    q1 = qkf[:, bi * 256 + 128: bi * 256 + 256]
                    k0 = qkf[:, 1024 + bi * 256: 1024 + bi * 256 + 128]
                    k1 = qkf[:, 1024 + bi * 256 + 128: 1024 + bi * 256 + 256]
                    w01 = wt[bi // 2]
                    c0 = (bi % 2) * 256
                    nc.tensor.matmul(w01[:, c0:c0 + 128], k0, q0,
                                     start=True, stop=False)
                    nc.tensor.matmul(w01[:, c0:c0 + 128], id128[:], wsut[:],
                                     start=False, stop=True)
                    nc.tensor.matmul(w01[:, c0 + 128:c0 + 256], k0, q1,
                                     start=True, stop=True)
                    c1 = bi * 128
                    nc.tensor.matmul(wt[2][:, c1:c1 + 128], k1, q1,
                                     start=True, stop=False)
                    nc.tensor.matmul(wt[2][:, c1:c1 + 128], id128[:], wsut[:],
                                     start=False, stop=True)
                st[m]["wt"] = wt

            def stage2(m):
                wt = st[m].pop("wt")
                e = wkp.tile([128, 1536], bf16, tag="e")
                for i in range(3):
                    nc.scalar.activation(e[:, i * 512:(i + 1) * 512],
                                         wt[i][:], AF.Exp)
                st[m]["e"] = e

            def stage3(m):
                o, j = divmod(m, 2)
                _, m0, m1 = io[o]
                e = st[m]["e"]
                a0 = wkp.tile([128, 1024], bf16, tag="a0")
                nc.gpsimd.tensor_mul(a0[:, 0:512], e[:, 0:512],
                                     m0[:, j * 1024: j * 1024 + 512])
                nc.gpsimd.tensor_mul(a0[:, 512:1024], e[:, 512:1024],
                                     m0[:, j * 1024 + 512: j * 1024 + 1024])
                a1 = wkp.tile([128, 512], bf16, tag="a1")
                nc.vector.tensor_mul(a1[:], e[:, 1024:1536],
                                     m1[:, j * 512: j * 512 + 512])
                st[m].update(a0=a0, a1=a1)

            def stage4(m):
                o, j = divmod(m, 2)
                s = st.pop(m)
                e, a0, a1, v2 = s["e"], s["a0"], s["a1"], s["v2"]
                zq = pp_z.tile([128, 8], f32, tag="pz")
                for bl in range(4):
                    nc.tensor.matmul(zq[:, bl:bl + 1],
                                     e[:, bl * 256: bl * 256 + 128],
                                     ones1[:], start=True, stop=True)
                    nc.tensor.matmul(zq[:, 4 + bl: 5 + bl],
                                     e[:, bl * 256 + 128: bl * 256 + 256],
                                     ones1[:], start=True, stop=False)
                    nc.tensor.matmul(zq[:, 4 + bl: 5 + bl],
                                     e[:, 1024 + bl * 128:
                                       1024 + bl * 128 + 128],
                                     ones1[:], start=False, stop=True)
                otq = pp_out.tile([128, 512], f32, tag="po")
                for bl in range(4):
                    vs0 = v2[:, bl * 128: bl * 128 + 64]
                    vs1 = v2[:, bl * 128 + 64: bl * 128 + 128]
                    nc.tensor.matmul(otq[:, bl * 64: bl * 64 + 64],
                                     a0[:, bl * 256: bl * 256 + 128],
                                     vs0, start=True, stop=True)
                    nc.tensor.matmul(otq[:, 256 + bl * 64: 256 + bl * 64 + 64],
                                     a0[:, bl * 256 + 128: bl * 256 + 256],
                                     vs0, start=True, stop=False)
                    nc.tensor.matmul(otq[:, 256 + bl * 64: 256 + bl * 64 + 64],
                                     a1[:, bl * 128: bl * 128 + 128],
                                     vs1, start=False, stop=True)
                izf = wkp.tile([128, 8], f32, tag="izf")
                nc.vector.reciprocal(izf[:], zq[:])
                of = wkp.tile([128, 512], bf16, tag="of")
                izb = izf[:].rearrange("p (v b o) -> p v b o", v=2, o=1) \
                    .broadcast_to([128, 2, 4, H])
                nc.vector.tensor_mul(
                    of[:].rearrange("p (v b h) -> p v b h", v=2, b=4),
                    otq[:].rearrange("p (v b h) -> p v b h", v=2, b=4), izb)
                nc.scalar.dma_start(
                    y_d[o, :, :, j * 4 * H:(j * 4 + 4) * H]
                    .rearrange("v p c -> p v c"),
                    of[:].rearrange("p (v c) -> p v c", v=2))

            # ---- pipelined emission (ready-first order) -------------------
            def run(tag, fn_, *a):
                n0 = len(nc.inst_map)
                fn_(*a)
                for nm_ in list(nc.inst_map.keys())[n0:]:
                    _LABELS[nm_] = tag

            run("dma(0)", dma_oct, 0)
            run("dma(1)", dma_oct, 1)
            for m in range(NG + 5):
                if m >= 5:
                    run(f"S4({m - 5})", stage4, m - 5)
                if 4 <= m < NG + 4:
                    run(f"S3({m - 4})", stage3, m - 4)
                if 3 <= m < NG + 3:
                    run(f"S2({m - 3})", stage2, m - 3)
                if 1 <= m < NG + 1:
                    run(f"S0b({m - 1})", stage0b, m - 1)
                if 2 <= m < NG + 2:
                    run(f"S1({m - 2})", stage1, m - 2)
                if m % 2 == 1 and (m + 3) // 2 < NO:
                    run(f"dma({(m + 3) // 2})", dma_oct, (m + 3) // 2)
                if m < NG:
                    run(f"S0({m})", stage0, m)
    nc.finalize()
    return nc


def _get_program():
    if "nc" not in _CACHE:
        _CACHE["nc"] = _build_program()
    return _CACHE["nc"]


def _to_bf16(a):
    import ml_dtypes
    return a.astype(ml_dtypes.bfloat16)


def kernel(**inputs):
    from concourse.bass_utils import run_bass_kernel_spmd

    x = np.asarray(inputs["x"], dtype=np.float32)
    wq = np.ascontiguousarray(inputs["Wq"], dtype=np.float32)
    wk = np.ascontiguousarray(inputs["Wk"], dtype=np.float32)
    wv = np.ascontiguousarray(inputs["Wv"], dtype=np.float32)
    mask = np.asarray(inputs["dropout_mask"], dtype=np.float32)

    wqk = np.empty((2, 128, 128), np.float32)
    wqk[:, :, 0:64] = (wq * SCALE).reshape(2, 128, 64)
    wqk[:, :, 64:128] = wk.reshape(2, 128, 64)
    wqk = _to_bf16(wqk)
    wvp = _to_bf16((wv / (1.0 - P_DROP)).reshape(2, 128, H))

    xT = x.transpose(0, 2, 1)
    xq = _to_bf16(
        xT.reshape(N_CORES, NO, OB, 2, 128, T).transpose(0, 1, 3, 4, 2, 5))
    xq = np.ascontiguousarray(xq)

    keepT = (mask >= P_DROP).transpose(0, 2, 1)
    m0 = _to_bf16(
        keepT[:, 0:128, :].reshape(N_CORES, NO, OB, 128, T)
        .transpose(0, 1, 3, 2, 4))
    m0 = np.ascontiguousarray(m0)
    m1 = keepT[:, 128:256, 128:256].reshape(N_CORES, NO, OB, 128, 128) \
        .transpose(0, 1, 3, 2, 4).astype(np.uint8)
    m1 = np.ascontiguousarray(m1)

    cst = np.zeros((3, 128, 128), np.float32)
    ii, jj = np.mgrid[0:128, 0:128]
    cst[0] = np.where(jj < ii, NEG, 0.0)          # wsut
    cst[1] = (ii == jj).astype(np.float32)        # id128
    cst[2] = 1.0                                  # ones
    cst = _to_bf16(cst)

    nc = _get_program()
    in_maps = [
        {"xq": xq[i], "m0": m0[i], "m1": m1[i], "wqk": wqk, "wv": wvp,
         "cst": cst}
        for i in range(N_CORES)
    ]
    res = run_bass_kernel_spmd(nc, in_maps, core_ids=list(range(N_CORES)))
    outs = []
    for r in res.results:
        yr = np.asarray(r["y"], dtype=np.float32).reshape(NO, 2, 128, OB, H)
        outs.append(np.ascontiguousarray(
            yr.transpose(0, 3, 1, 2, 4)).reshape(BP, T, H))
    return np.concatenate(outs, axis=0).astype(np.float32)
